# revision 3
# baseline (speedup 1.0000x reference)
"""Trainium2 Bass kernel for nn_BaseSearchBasedModel (sparse attention), v3.

Math (exact up to fp rounding), per sample s with T = topk rows [K, D]:
  scores = T @ (A^T tgt + c) / .. ;  A = WQ WK^T / 8, c = WK bQ / 8
    (the q.bK term is constant over the softmax axis -> drops out)
  attn   = softmax(scores)
  ctx    = T^T @ attn                (WV folded after softmax)
  mhta   = sum_h ctx[h] @ G[h] + bias0 ;  G = WV[h] WO_h, bias0 = sum_h bV[h] WO_h + bO
  logit  = MLP(concat(mhta, tgt))

Sharding: pure data parallel, batch 2048 -> 8 cores x 256.

v6 = v5 + ktab in fp8 e3m4 (4 mantissa bits, range +-15.9 -- ample for
randn inputs; e4m3 measured 1.86e-2 rel err, e3m4 should halve that):
  - chunk cadence: v3 ran ~2.3us/chunk because the in-order DVE queue
    head-of-line blocked on the z->reciprocal->broadcast chain, and the
    shared score/ctx PSUM pool coupled that chain back into the next
    chunk's scores. Now: scores are emitted one chunk AHEAD, the
    normalize multiplies one chunk LATE, sc/ctx use separate PSUM pools,
    and 1/z uses the single-op reciprocal_approx_fast (~212ns).
  - the first half of the G-projection accumulates mid-loop (after chunk
    8); b2 is folded into the last matmul via a ones-row-augmented h1,
    so the logit copy is a plain DVE copy.
"""

import sys

if "/opt/trn_rl_repo" not in sys.path:
    sys.path.insert(0, "/opt/trn_rl_repo")

import numpy as np

import concourse.bass as bass
import concourse.tile as tile
import concourse.mybir as mybir
from concourse import bacc
from concourse.bass_utils import run_bass_kernel_spmd

F32 = mybir.dt.float32
BF16 = mybir.dt.bfloat16
FP8E3 = mybir.dt.float8e3
AF = mybir.ActivationFunctionType

B, L, K, D, H = 2048, 1024, 128, 64, 4
N_CORES = 8
B_LOC = B // N_CORES  # 256

# wtab column map (bf16, 128 rows)
W1_C, G_C, W2_C = 0, 64, 320
WTAB_C = 321
# fblob column map (f32, 64 rows)
BIAS0_C, B1_C, B2_C = 0, 1, 2
FBLOB_C = 3


def build(b_loc=B_LOC):
    assert b_loc % 32 == 0
    n_chunk = b_loc // 16        # 16
    n_pair = b_loc // 2          # 128
    n_grp = n_chunk // 2         # tab DMA groups (2 chunks each)

    nc = bacc.Bacc("TRN2", target_bir_lowering=False, debug=False,
                   num_devices=N_CORES)

    ptab = nc.dram_tensor("ptab", [128, n_pair * K], BF16, kind="ExternalInput").ap()
    ktab = nc.dram_tensor("ktab", [128, n_pair * K], FP8E3, kind="ExternalInput").ap()
    ablob = nc.dram_tensor("ablob", [65, 2 * H * D], BF16, kind="ExternalInput").ap()
    wtab = nc.dram_tensor("wtab", [128, WTAB_C], BF16, kind="ExternalInput").ap()
    fblob = nc.dram_tensor("fblob", [D, FBLOB_C], F32, kind="ExternalInput").ap()
    xtab = nc.dram_tensor("xtab", [128, b_loc], BF16, kind="ExternalInput").ap()
    logit = nc.dram_tensor("logit", [b_loc, 1], F32, kind="ExternalOutput").ap()

    grp_cols = 2 * 8 * K  # 2048 cols per 2-chunk DMA group

    with tile.TileContext(nc) as tc, \
         tc.tile_pool(name="const", bufs=1) as const, \
         tc.tile_pool(name="expsb", bufs=4) as exp_pool, \
         tc.tile_pool(name="rz", bufs=4) as rz_pool, \
         tc.tile_pool(name="scps", bufs=2, space="PSUM") as sc_pool, \
         tc.tile_pool(name="ctxps", bufs=3, space="PSUM") as ctx_pool, \
         tc.tile_pool(name="zps", bufs=1, space="PSUM") as z_pool, \
         tc.tile_pool(name="qkps", bufs=1, space="PSUM") as qk_pool, \
         tc.tile_pool(name="endps", bufs=1, space="PSUM") as end_pool:

        # ---- input DMAs: all on the Sync queue, blobs first ----
        ab = const.tile([65, 2 * H * D], BF16, tag="ab")
        nc.sync.dma_start(out=ab[:], in_=ablob[:])
        wt = const.tile([128, WTAB_C], BF16, tag="wt")
        nc.sync.dma_start(out=wt[:], in_=wtab[:])
        fb = const.tile([D, FBLOB_C], F32, tag="fb")
        nc.sync.dma_start(out=fb[:], in_=fblob[:])
        xT = const.tile([128, b_loc], BF16, tag="xT")
        nc.sync.dma_start(out=xT[:], in_=xtab[:])
        ptab_g = []
        ktab_g = []
        for g in range(n_grp):
            pt = const.tile([128, grp_cols], BF16, tag=f"ptab{g}")
            nc.sync.dma_start(out=pt[:], in_=ptab[:, grp_cols * g:grp_cols * (g + 1)])
            ptab_g.append(pt)
            kt = const.tile([128, grp_cols], FP8E3, tag=f"ktab{g}")
            nc.sync.dma_start(out=kt[:], in_=ktab[:, grp_cols * g:grp_cols * (g + 1)])
            ktab_g.append(kt)

        ones_col16 = const.tile([128, 1], BF16, tag="ones_col16")
        nc.vector.memset(ones_col16[:], 1.0)
        h1_aug = const.tile([D + 1, b_loc], BF16, tag="h1aug")
        nc.vector.memset(h1_aug[D:D + 1, :], 1.0)

        A65 = ab[:, 0:H * D]
        eo65 = ab[:, H * D:2 * H * D]
        w1_sb = wt[:, W1_C:W1_C + D]
        G_sb = wt[:, G_C:G_C + H * D]
        w2c65 = wt[:, W2_C:W2_C + 1][0:D + 1, :]
        bias0 = fb[:, BIAS0_C:BIAS0_C + 1]
        b1col = fb[:, B1_C:B1_C + 1]
        b2v = fb[0:1, B2_C:B2_C + 1]

        # ---- qk block-diagonal tile [128, 8*n_pair] (tgt-dependent) ----
        # pair q cols 8q..8q+7: cols 0-3 = even-sample heads (rows 0:64),
        # cols 4-7 = odd (rows 64:128); rest zero. Bias c rides contraction
        # row 64 of A65/eo65.
        qk_bd = const.tile([128, 8 * n_pair], BF16, tag="qk_bd")
        nc.vector.memset(qk_bd[:], 0.0)
        qk_v = qk_bd[:].rearrange("p (q c) -> p q c", c=8)
        for h in range(H):
            qk_ps = qk_pool.tile([128, n_pair], F32, tag="qkps")
            nc.tensor.matmul(qk_ps[0:D, :], lhsT=A65[:, D * h:D * (h + 1)],
                             rhs=eo65[:, 0:n_pair], start=True, stop=True)
            nc.tensor.matmul(qk_ps[D:2 * D, :], lhsT=A65[:, D * h:D * (h + 1)],
                             rhs=eo65[:, n_pair:2 * n_pair], start=True, stop=True)
            nc.vector.tensor_copy(out=qk_v[0:D, :, h], in_=qk_ps[0:D, :])
            nc.vector.tensor_copy(out=qk_v[D:2 * D, :, 4 + h], in_=qk_ps[D:2 * D, :])

        # ---- main loop over chunks of 16 samples (8 pairs) ----
        # The normalize multiplies for chunk c are emitted during chunk
        # c+1 (and the G-projection first half after chunk 8) so the
        # in-order DVE/PE queues never stall on the current chunk's
        # exp -> z -> reciprocal -> broadcast chain.
        ctxn = const.tile([128, 2 * b_loc], BF16, tag="ctxn")
        ctxn_v = ctxn[:].rearrange("p (c cc q) -> p c cc q", c=4, cc=n_chunk)
        ctxn_h = ctxn[:].rearrange("p (c s) -> p c s", c=4)
        mh2 = end_pool.tile([128, n_pair], F32, tag="end")

        def emit_mults(c, ctx_ps, rzb):
            ctx_v = ctx_ps[:].rearrange("p (q c) -> p c q", c=8)
            rzb_v = rzb[:].rearrange("p (q c) -> p c q", c=8)
            nc.vector.tensor_tensor(out=ctxn_v[0:D, :, c, :], in0=ctx_v[0:D, 0:4, :],
                                    in1=rzb_v[0:D, 0:4, :], op=mybir.AluOpType.mult)
            nc.vector.tensor_tensor(out=ctxn_v[D:2 * D, :, c, :],
                                    in0=ctx_v[D:2 * D, 4:8, :],
                                    in1=rzb_v[D:2 * D, 4:8, :],
                                    op=mybir.AluOpType.mult)

        def emit_mh(lo, hi, first, last):
            for h in range(H):
                nc.tensor.matmul(mh2[0:D, lo:hi], lhsT=G_sb[0:D, D * h:D * (h + 1)],
                                 rhs=ctxn_h[0:D, h, lo:hi],
                                 start=(h == 0) and first, stop=(h == H - 1) and last)
            for h in range(H):
                nc.tensor.matmul(mh2[D:2 * D, lo:hi],
                                 lhsT=G_sb[D:2 * D, D * h:D * (h + 1)],
                                 rhs=ctxn_h[D:2 * D, h, lo:hi],
                                 start=(h == 0) and first, stop=(h == H - 1) and last)

        def emit_scores(c):
            g, half = c // 2, c % 2
            pwin = ptab_g[g][:, half * 1024:half * 1024 + 1024]
            sc_ps = sc_pool.tile([128, 64], F32, tag="sc")
            for q in range(8):
                nc.tensor.matmul(sc_ps[:, 8 * q:8 * (q + 1)],
                                 lhsT=pwin[:, K * q:K * (q + 1)],
                                 rhs=qk_bd[:, 64 * c + 8 * q:64 * c + 8 * (q + 1)],
                                 start=True, stop=True)
            return sc_ps

        pending = None
        sc_ps = emit_scores(0)
        for c in range(n_chunk):
            g, half = c // 2, c % 2
            kwin = ktab_g[g][:, half * 1024:half * 1024 + 1024]
            next_sc = emit_scores(c + 1) if c + 1 < n_chunk else None
            exp_sb = exp_pool.tile([128, 64], BF16, tag="exp")
            nc.scalar.activation(exp_sb[:], sc_ps[:], AF.Exp)
            sc_ps = next_sc
            z_ps = z_pool.tile([1, 64], F32, tag="z")
            nc.tensor.matmul(z_ps[:], lhsT=ones_col16[:], rhs=exp_sb[:],
                             start=True, stop=True)
            rz = rz_pool.tile([1, 64], F32, tag="rz")
            nc.vector.reciprocal_approx_fast(out=rz[:], in_=z_ps[:])
            rzb = rz_pool.tile([128, 64], F32, tag="rzb")
            nc.gpsimd.partition_broadcast(rzb[:], rz[:])
            ctx_ps = ctx_pool.tile([128, 64], F32, tag="ctx")
            for q in range(8):
                nc.tensor.matmul(ctx_ps[:, 8 * q:8 * (q + 1)],
                                 lhsT=kwin[:, K * q:K * (q + 1)],
                                 rhs=exp_sb[:, 8 * q:8 * (q + 1)], start=True, stop=True)
            if pending is not None:
                emit_mults(*pending)
            pending = (c, ctx_ps, rzb)
            if c == 8:
                emit_mh(0, 64, True, False)
        emit_mults(*pending)
        emit_mh(64, n_pair, False, True)

        # ---- bias + MLP (bf16 activations) ----
        x_v = xT[:].rearrange("p (s two) -> p s two", two=2)
        nc.scalar.activation(x_v[0:D, :, 0], mh2[0:D, :], AF.Identity,
                             bias=bias0[:], scale=1.0)
        nc.scalar.activation(x_v[0:D, :, 1], mh2[D:2 * D, :], AF.Identity,
                             bias=bias0[:], scale=1.0)
        h1_ps = end_pool.tile([D, b_loc], F32, tag="end")
        nc.tensor.matmul(h1_ps[:], lhsT=w1_sb[:], rhs=xT[:], start=True, stop=True)
        nc.scalar.activation(h1_aug[0:D, :], h1_ps[:], AF.Relu, bias=b1col[:],
                             scale=1.0)
        lg_ps = end_pool.tile([1, b_loc], F32, tag="end")
        nc.tensor.matmul(lg_ps[:], lhsT=w2c65[:], rhs=h1_aug[:], start=True, stop=True)
        lg_sb = const.tile([1, b_loc], F32, tag="lg")
        nc.vector.tensor_copy(out=lg_sb[:], in_=lg_ps[:])
        nc.sync.dma_start(out=logit[:], in_=lg_sb[:])

    nc.compile()
    return nc


def make_in_maps(inputs, b_loc=B_LOC, n_cores=N_CORES):
    """Shard + prep: host gather into both matmul orientations, weight folds."""
    import ml_dtypes
    BF = ml_dtypes.bfloat16

    idx = np.asarray(inputs["indices"]).astype(np.int64)
    useq = np.asarray(inputs["user_seq_emb"], dtype=np.float32)
    tgt = np.asarray(inputs["target_emb"], dtype=np.float32)[:, 0, :]  # [B, D]
    WQ = np.asarray(inputs["WQ"], np.float32)
    WK = np.asarray(inputs["WK"], np.float32)
    WV = np.asarray(inputs["WV"], np.float32)
    bQ = np.asarray(inputs["bQ"], np.float32)
    bV = np.asarray(inputs["bV"], np.float32)
    WO = np.asarray(inputs["WO"], np.float32)
    bO = np.asarray(inputs["bO"], np.float32)
    W1 = np.asarray(inputs["W1"], np.float32)
    b1 = np.asarray(inputs["b1"], np.float32)
    W2 = np.asarray(inputs["W2"], np.float32)
    b2 = np.asarray(inputs["b2"], np.float32)

    # host gather (the sharding prep): [B, K, D] -> bf16
    topk = np.take_along_axis(useq, idx[:, :, None], axis=1).astype(BF)

    # weight folds
    A = np.stack([WQ[h] @ WK[h].T for h in range(H)], 0) / 8.0      # [H, D, D]
    c = np.stack([WK[h] @ bQ[h] for h in range(H)], 1) / 8.0        # [D, H]
    WO_h = WO.reshape(H, D, D)
    G = np.stack([WV[h] @ WO_h[h] for h in range(H)], 0)            # [H, D, D]
    bias0 = np.einsum("hd,hdf->f", bV, WO_h) + bO                   # [D]

    wtab = np.zeros((128, WTAB_C), dtype=BF)
    wtab[:2 * D, W1_C:W1_C + D] = W1.astype(BF)
    Gflat = G.transpose(1, 0, 2).reshape(D, H * D)                  # [e, (h, f)]
    wtab[0:D, G_C:G_C + H * D] = Gflat.astype(BF)
    wtab[D:2 * D, G_C:G_C + H * D] = Gflat.astype(BF)
    wtab[0:D, W2_C] = W2[:, 0].astype(BF)
    wtab[D, W2_C] = BF(b2[0])

    fblob = np.zeros((D, FBLOB_C), dtype=np.float32)
    fblob[:, BIAS0_C] = bias0
    fblob[:, B1_C] = b1
    fblob[0, B2_C] = b2[0]

    # A-blob with the c-bias as contraction row 64
    A65 = np.zeros((65, H * D), dtype=BF)
    A65[0:D, :] = A.transpose(1, 0, 2).reshape(D, H * D).astype(BF)  # [d, (h, e)]
    A65[D, :] = c.T.reshape(H * D).astype(BF)                        # c[e, h] -> row

    n_chunk = b_loc // 16
    in_maps = []
    for ci in range(n_cores):
        s = slice(ci * b_loc, (ci + 1) * b_loc)
        t = np.asarray(topk[s])                                     # [256, K, D] bf16
        tt = t.reshape(n_chunk, 8, 2, K, D)                         # [c, q, e, k, d]
        ktab = np.ascontiguousarray(
            tt.transpose(3, 0, 1, 2, 4).reshape(K, b_loc // 2 * K)
            .astype(ml_dtypes.float8_e3m4))                          # [k][(c q e d)]
        ptab = np.ascontiguousarray(
            tt.transpose(2, 4, 0, 1, 3).reshape(K, b_loc // 2 * K))  # [(e d)][(c q k)]
        tg = tgt[s]                                                  # [256, D]
        tgtT = tg.T                                                  # [D, 256]
        eo65 = np.ones((65, b_loc), dtype=BF)
        eo65[0:D, 0:b_loc // 2] = tgtT[:, 0::2].astype(BF)
        eo65[0:D, b_loc // 2:] = tgtT[:, 1::2].astype(BF)
        ab = np.concatenate([A65, eo65], axis=1)
        xtab = np.zeros((128, b_loc), dtype=BF)
        xtab[D:2 * D, :] = tgtT.astype(BF)
        in_maps.append({
            "ptab": ptab, "ktab": ktab, "ablob": np.ascontiguousarray(ab),
            "wtab": wtab, "fblob": fblob, "xtab": xtab,
        })
    return in_maps


_NC_CACHE = {}


def kernel(**inputs):
    if B_LOC not in _NC_CACHE:
        _NC_CACHE[B_LOC] = build(B_LOC)
    nc = _NC_CACHE[B_LOC]
    in_maps = make_in_maps(inputs)
    res = run_bass_kernel_spmd(nc, in_maps, core_ids=list(range(N_CORES)))
    return np.concatenate([res.results[c]["logit"] for c in range(N_CORES)], axis=0)
